# revision 1
# baseline (speedup 1.0000x reference)
"""Trainium2 Bass kernel for a 2-layer categorical GIN encoder.

Graph: N=100000 nodes, E=1600000 edges, 256-dim features.

    x   = concat_i emb_i[x_cat[:, i]]                  # [N, 256]
    h1  = LN1(relu(relu((x + A x) @ w1a + b1a) @ w1b + b1b))
    out = LN2(relu((h1 + A h1) @ w2a + b2a) @ w2b + b2b)

where (A x)[d] = sum over edges (s -> d) of x[s].

Strategy (8 NeuronCores, SPMD):
  * Linearity trick: (x + Ax) @ w1a == z + A z with z = x @ w1a, and
    z[n] = sum_i t_i[x_cat[n, i]] where t_i = emb_i @ w1a[64i:64i+64, :].
    Layer 1 never materializes x: each core builds its shard of the
    bf16 z table by gathering rows of the small t tables (built on
    device), then the shards are AllGathered (HBM-HBM, Shared output)
    so every core holds the full z table.
  * Edges are sorted by destination on the host and sharded by
    destination-node range (12500 nodes per core) -> no collective is
    needed for the aggregation itself; each core owns its rows.
  * Per 128-destination-node tile, edges are processed in chunks of
    128 (chunk count fixed per tile = max over cores, so the single
    SPMD program is shape-uniform): one indirect DMA per chunk gathers
    z[src] (512B bf16 rows), a one-hot selection matrix
    S[e, d] = (dst_rel[e] == d) is built on DVE by comparing against an
    iota row, and agg += S.T @ gathered accumulates in PSUM on the
    tensor engine (bf16 in, fp32 accumulate).  Padding edges point at
    row 0 with dst_rel = -1, so they gather real data but select zero.
  * The per-tile MLP runs on-chip: PE transposes (via identity matmul)
    put activations feature-major for the bf16 weight matmuls;
    LayerNorm uses bn_stats/bn_aggr + Sqrt/reciprocal.  Layer 1 also
    applies w2a immediately so the second AllGather ships
    z2 = h1 @ w2a, and layer 2 repeats the same aggregation + MLP
    pipeline to produce the fp32 output shard.
  * Measured bottleneck is GPSIMD SWDGE descriptor generation for the
    per-edge gathers (~1.1us per 128-row indirect DMA); PE/DVE/DMA all
    run below that envelope, hence bf16 tables + per-tile chunk counts.

Host-side work is limited to index manipulation (edge sort / layout,
concatenated-table index offsets) and small layout transforms; all
floating-point math runs on the NeuronCores.
"""

import numpy as np

# ---------------------------------------------------------------------------
# Problem constants (hardcoded per contest contract)
# ---------------------------------------------------------------------------
N = 100000        # nodes
E = 1600000       # edges
D = 256           # feature dim (in = hidden = out)
EMB = 64          # per-field embedding dim
V = 1000          # categories per field
NF = 4            # categorical fields
NC = 8            # NeuronCores
P = 128           # partitions
LN_EPS = 1e-5

NSH = N // NC             # nodes per core (12500)
NT = (NSH + P - 1) // P   # node tiles per core (98)
LAST_VALID = NSH - (NT - 1) * P  # valid rows in last tile (84)


# ---------------------------------------------------------------------------
# Host-side preprocessing: shard + sort edges, build per-core metadata
# ---------------------------------------------------------------------------
def _prep_meta(x_cat, edge_index, nsh=NSH, nt=NT, num_cores=NC, n_slices=1):
    """Returns (Ct, slice_lens, cmeta[NC], src_meta[NC], dstrel_meta[NC]).

    Ct[t] = chunks for tile t (max over cores).  The full z/z2 tables are
    laid out slice-major (slice, core, row) so AllGathers can be issued
    per slice; src indices are pre-permuted into that layout.
    """
    n_slices = max(1, min(n_slices, nt))
    src = np.asarray(edge_index[0], dtype=np.int64)
    dst = np.asarray(edge_index[1], dtype=np.int64)

    order = np.argsort(dst, kind="stable")
    dst_s = dst[order]
    src_s = src[order]

    bounds = np.searchsorted(dst_s, np.arange(num_cores + 1) * nsh)

    per_core = []
    all_counts = np.zeros((num_cores, nt), dtype=np.int64)
    for k in range(num_cores):
        lo, hi = bounds[k], bounds[k + 1]
        d_k = dst_s[lo:hi] - k * nsh
        s_k = src_s[lo:hi]
        t_k = d_k // P
        rel = (d_k - t_k * P).astype(np.float32)
        counts = np.bincount(t_k, minlength=nt)
        all_counts[k] = counts
        per_core.append((s_k, t_k, rel, counts))
    Ct = np.maximum(1, -(-all_counts.max(axis=0) // P))      # [nt]
    coff = np.zeros(nt + 1, dtype=np.int64)
    np.cumsum(Ct, out=coff[1:])
    CT = int(coff[-1])

    # slice-major permutation of full-table rows
    tile_groups = np.array_split(np.arange(nt), n_slices)
    slice_lens = []
    row_base = []  # per-slice starting row within a shard
    rb = 0
    for g in tile_groups:
        lo_r = rb
        lr = 0
        for t in g:
            lr += (nsh - t * P) if t == nt - 1 else P
        slice_lens.append(lr)
        row_base.append(lo_r)
        rb += lr
    assert rb == nsh
    glob_off = np.zeros(n_slices + 1, dtype=np.int64)
    np.cumsum(np.array(slice_lens) * num_cores, out=glob_off[1:])
    sl_of_row = np.searchsorted(np.cumsum(slice_lens), np.arange(nsh),
                                side="right")     # [nsh] -> slice id

    def permute(node):
        k = node // nsh
        r = node - k * nsh
        s = sl_of_row[r]
        return (glob_off[s] + k * np.take(slice_lens, s)
                + (r - np.take(row_base, s)))

    src_meta, dstrel_meta, cmeta = [], [], []
    x_cat = np.asarray(x_cat, dtype=np.int64)
    field_off = (np.arange(NF) * V)[None, :]
    for k in range(num_cores):
        s_k, t_k, rel, counts = per_core[k]
        starts = np.zeros(nt, dtype=np.int64)
        np.cumsum(counts[:-1], out=starts[1:])
        j = np.arange(len(s_k)) - starts[t_k]
        col = coff[t_k] + j // P
        row = (j % P).astype(np.int64)

        sm = np.zeros((P, CT), dtype=np.int32)
        dm = np.full((P, CT), -1.0, dtype=np.float32)
        sm[row, col] = permute(s_k).astype(np.int32)
        dm[row, col] = rel
        src_meta.append(sm)
        dstrel_meta.append(dm)

        cm = np.zeros((nt * P, NF), dtype=np.int32)
        xc = x_cat[k * nsh:(k + 1) * nsh] + field_off
        cm[:nsh] = xc.astype(np.int32)
        cmeta.append(cm.reshape(nt, P, NF))
    return ([int(c) for c in Ct], slice_lens, cmeta, src_meta, dstrel_meta)


# ---------------------------------------------------------------------------
# Device program
# ---------------------------------------------------------------------------
_PROGRAM_CACHE = {}


def _build_program(Ct, slice_lens, n=N, nsh=NSH, nt=NT,
                   last_valid=LAST_VALID,
                   use_biases=False, use_ln_gb=False, num_cores=NC,
                   mm_dt_name="bfloat16"):
    """Build + compile the SPMD Bass program.  Returns (nc, input_names)."""
    import concourse.bacc as bacc
    import concourse.bass as bass
    import concourse.tile as tile
    from concourse import mybir

    f32 = mybir.dt.float32
    i32 = mybir.dt.int32
    mm_dt = getattr(mybir.dt, mm_dt_name)

    nc = bacc.Bacc("TRN2", target_bir_lowering=False, debug=False,
                   num_devices=num_cores)

    # ---- external I/O ----
    embT_d = nc.dram_tensor("embT", [NF, EMB, V], f32, kind="ExternalInput")
    w1a_d = nc.dram_tensor("w1a", [D, D], f32, kind="ExternalInput")
    w1b_d = nc.dram_tensor("w1b", [D, D], f32, kind="ExternalInput")
    w2a_d = nc.dram_tensor("w2a", [D, D], f32, kind="ExternalInput")
    w2b_d = nc.dram_tensor("w2b", [D, D], f32, kind="ExternalInput")
    cmeta_d = nc.dram_tensor("cmeta", [nt, P, NF], i32, kind="ExternalInput")
    coff = [0]
    for c in Ct:
        coff.append(coff[-1] + c)
    CT = coff[-1]
    Cmax = max(Ct)
    n_slices = len(slice_lens)
    sl_rows = [0]
    for L in slice_lens:
        sl_rows.append(sl_rows[-1] + L)
    glob_off = [0]
    for L in slice_lens:
        glob_off.append(glob_off[-1] + L * num_cores)
    # tile group boundaries per slice (tiles are split the same way)
    import numpy as _np
    tile_groups = _np.array_split(_np.arange(nt), n_slices)
    last_tile_of_slice = {int(g[-1]): i for i, g in enumerate(tile_groups)}
    srcm_d = nc.dram_tensor("src_meta", [P, CT], i32, kind="ExternalInput")
    drel_d = nc.dram_tensor("dstrel_meta", [P, CT], f32, kind="ExternalInput")
    iota_d = nc.dram_tensor("iota_row", [P, P], f32, kind="ExternalInput")
    ident_d = nc.dram_tensor("identity", [P, P], f32, kind="ExternalInput")
    bias_d = None
    if use_biases or use_ln_gb:
        # rows: b1a, b1b, b2a, b2b, ln1_g, ln1_b, ln2_g, ln2_b
        bias_d = nc.dram_tensor("biasrows", [8, D], f32, kind="ExternalInput")
    out_d = nc.dram_tensor("out", [nsh, D], f32, kind="ExternalOutput")

    groups = [list(range(num_cores))]

    from contextlib import ExitStack

    with tile.TileContext(nc) as tc, ExitStack() as ctx:
        singles = ctx.enter_context(tc.tile_pool(name="singles", bufs=1))
        dram = ctx.enter_context(tc.tile_pool(name="dram", bufs=1, space="DRAM"))
        meta_p = ctx.enter_context(tc.tile_pool(name="meta", bufs=8))
        gath_p = ctx.enter_context(tc.tile_pool(name="gath", bufs=6))
        sel_p = ctx.enter_context(tc.tile_pool(name="sel", bufs=4))
        work_p = ctx.enter_context(tc.tile_pool(name="work", bufs=4))
        stat_p = ctx.enter_context(tc.tile_pool(name="stat", bufs=4))
        ps_agg = ctx.enter_context(tc.tile_pool(name="ps_agg", bufs=3, space="PSUM"))
        ps_tr = ctx.enter_context(tc.tile_pool(name="ps_tr", bufs=2, space="PSUM"))
        ps_mm = ctx.enter_context(tc.tile_pool(name="ps_mm", bufs=2, space="PSUM"))

        # ---- internal DRAM tables (mm_dt so fp32r matmuls see rounded
        # producers; storage is still 4 bytes for float32r) ----
        t_dram = dram.tile([NF * V, D], f32)                       # t tables
        z_shard = dram.tile([nsh, D], mm_dt)
        z_full = dram.tile([n, D], mm_dt,
                           addr_space="Shared" if n_slices == 1 else "Local")
        z2_shard = dram.tile([nsh, D], mm_dt)
        z2_full = dram.tile([n, D], mm_dt,
                            addr_space="Shared" if n_slices == 1 else "Local")

        # ---- persistent SBUF constants ----
        iota_sb = singles.tile([P, P], f32)
        nc.sync.dma_start(out=iota_sb[:], in_=iota_d[:])
        ident_sb = singles.tile([P, P], f32)
        nc.sync.dma_start(out=ident_sb[:], in_=ident_d[:])
        eps_sb = singles.tile([P, 1], f32)
        nc.vector.memset(eps_sb[:], LN_EPS)

        def load_w(dram_t, name):
            tiles = []
            for kk in range(2):
                w_sb = singles.tile([P, D], f32, name=f"{name}_{kk}")
                nc.sync.dma_start(out=w_sb[:], in_=dram_t[kk * P:(kk + 1) * P, :])
                tiles.append(w_sb)
            return tiles

        w1a_rows = []
        for f in range(NF):
            w1a_r = singles.tile([EMB, D], f32, name=f"w1a_r{f}")
            nc.sync.dma_start(out=w1a_r[:], in_=w1a_d[f * EMB:(f + 1) * EMB, :])
            w1a_rows.append(w1a_r)
        def round_w(tiles, name):
            out = []
            for kk, w_sb in enumerate(tiles):
                w_r = singles.tile([P, D], mm_dt, name=f"{name}r_{kk}")
                nc.vector.tensor_copy(out=w_r[:], in_=w_sb[:])
                out.append(w_r)
            return out

        w1b_sb = round_w(load_w(w1b_d, "w1b"), "w1b")
        w2a_sb = round_w(load_w(w2a_d, "w2a"), "w2a")
        w2b_sb = round_w(load_w(w2b_d, "w2b"), "w2b")

        bias_sb = None
        if bias_d is not None:
            bias_tile = singles.tile([P, 8, D], f32)
            for r in range(8):
                nc.sync.dma_start(
                    out=bias_tile[:, r, :],
                    in_=bias_d[r].unsqueeze(0).to_broadcast([P, D]))
            bias_sb = [bias_tile[:, r, :] for r in range(8)]

        # =================================================================
        # Phase B: t tables  t[f] = emb_f @ w1a[64f:64f+64, :]   -> t_dram
        # =================================================================
        MT = 125  # 1000 = 8 * 125
        embT_p = ctx.enter_context(tc.tile_pool(name="embT_p", bufs=1))
        for f in range(NF):
            embT_sb = embT_p.tile([EMB, V], f32, tag="embT")
            nc.sync.dma_start(out=embT_sb[:], in_=embT_d[f])
            w_rows = w1a_rows[f][:]
            for j in range(V // MT):
                t_ps = ps_mm.tile([MT, D], f32, tag="tps", bufs=1)
                nc.tensor.matmul(out=t_ps[:],
                                 lhsT=embT_sb[:, j * MT:(j + 1) * MT],
                                 rhs=w_rows, start=True, stop=True)
                t_sb = work_p.tile([MT, D], f32, tag="tsb")
                nc.vector.tensor_copy(out=t_sb[:], in_=t_ps[:])
                nc.sync.dma_start(
                    out=t_dram[f * V + j * MT:f * V + (j + 1) * MT, :],
                    in_=t_sb[:])

        # =================================================================
        # Phase C: z shard  z[n] = sum_f t[cmeta[n, f]]          -> z_shard
        # =================================================================
        for t in range(nt):
            valid = last_valid if t == nt - 1 else P
            cm = meta_p.tile([P, NF], i32, tag="cm")
            nc.sync.dma_start(out=cm[:], in_=cmeta_d[t])
            g4 = gath_p.tile([P, NF, D], f32, tag="g4")
            for f in range(NF):
                nc.gpsimd.indirect_dma_start(
                    out=g4[:, f, :], out_offset=None,
                    in_=t_dram[:],
                    in_offset=bass.IndirectOffsetOnAxis(ap=cm[:, f:f + 1], axis=0))
            t01 = work_p.tile([P, 2, D], f32, tag="t01")
            nc.vector.tensor_add(out=t01[:], in0=g4[:, 0:2, :], in1=g4[:, 2:4, :])
            z_t = work_p.tile([P, D], mm_dt, tag="z_t")
            nc.vector.tensor_add(out=z_t[:], in0=t01[:, 0, :], in1=t01[:, 1, :])
            nc.sync.dma_start(out=z_shard[t * P:t * P + valid, :],
                              in_=z_t[:valid, :])

        for s in range(n_slices):
            nc.gpsimd.collective_compute(
                "AllGather", mybir.AluOpType.bypass, replica_groups=groups,
                ins=[z_shard[sl_rows[s]:sl_rows[s + 1], :]],
                outs=[z_full[glob_off[s]:glob_off[s + 1], :]])

        # =================================================================
        # Phases D/E: message passing + MLP layers
        # =================================================================
        def mp_layer(layer):
            """layer 1: gather z, produce z2 shard.  layer 2: gather z2,
            produce output shard."""
            tab_full = z_full if layer == 1 else z2_full
            tab_own = z_shard if layer == 1 else z2_shard
            wb_sb = w1b_sb if layer == 1 else w2b_sb
            ba_row, bb_row = (0, 1) if layer == 1 else (2, 3)
            g_row, b_row = (4, 5) if layer == 1 else (6, 7)

            for t in range(nt):
                valid = last_valid if t == nt - 1 else P
                C_t = Ct[t]
                c0 = coff[t]
                # --- metadata + gather ---
                srcm = meta_p.tile([P, Cmax], i32, tag="srcm")
                nc.sync.dma_start(out=srcm[:, :C_t], in_=srcm_d[:, c0:c0 + C_t])
                drel = meta_p.tile([P, Cmax], f32, tag="drel")
                nc.sync.dma_start(out=drel[:, :C_t], in_=drel_d[:, c0:c0 + C_t])
                G = gath_p.tile([P, Cmax, D], mm_dt, tag="G")
                for c in range(C_t):
                    nc.gpsimd.indirect_dma_start(
                        out=G[:, c, :], out_offset=None,
                        in_=tab_full[:],
                        in_offset=bass.IndirectOffsetOnAxis(ap=srcm[:, c:c + 1],
                                                            axis=0))
                # --- selection matrix for all chunks: S[e, c, d] ---
                S = sel_p.tile([P, Cmax, P], mm_dt, tag="S")
                nc.vector.tensor_tensor(
                    out=S[:, :C_t, :],
                    in0=drel[:, :C_t].unsqueeze(2).to_broadcast([P, C_t, P]),
                    in1=iota_sb[:].unsqueeze(1).to_broadcast([P, C_t, P]),
                    op=mybir.AluOpType.is_equal)
                # --- aggregate: agg[d, :] += S[:, c, d].T @ G[:, c, :] ---
                agg_ps = ps_agg.tile([P, D], f32, tag="agg")
                for c in range(C_t):
                    nc.tensor.matmul(out=agg_ps[:],
                                     lhsT=S[:, c, :], rhs=G[:, c, :],
                                     start=(c == 0), stop=(c == C_t - 1))
                # --- u = relu(z_own + agg (+ba)) ---
                zown = work_p.tile([P, D], mm_dt, tag="zown")
                if valid < P:
                    nc.vector.memset(zown[:], 0.0)
                nc.sync.dma_start(out=zown[:valid, :],
                                  in_=tab_own[t * P:t * P + valid, :])
                u = work_p.tile([P, D], f32, tag="u")
                nc.vector.tensor_add(out=u[:], in0=agg_ps[:], in1=zown[:])
                if use_biases:
                    nc.vector.tensor_add(out=u[:], in0=u[:], in1=bias_sb[ba_row])
                nc.vector.tensor_scalar_max(out=u[:], in0=u[:], scalar1=0.0)
                # --- v = u @ wb (+bb) ---
                uT_ps = ps_tr.tile([P, 2, P], f32, tag="uT_ps")
                for kk in range(2):
                    nc.tensor.transpose(out=uT_ps[:, kk, :],
                                        in_=u[:, kk * P:(kk + 1) * P],
                                        identity=ident_sb[:])
                uT = work_p.tile([P, 2, P], mm_dt, tag="uT")
                nc.vector.tensor_copy(out=uT[:], in_=uT_ps[:])
                v_ps = ps_mm.tile([P, D], f32, tag="v_ps")
                for kk in range(2):
                    nc.tensor.matmul(out=v_ps[:],
                                     lhsT=uT[:, kk, :], rhs=wb_sb[kk][:],
                                     start=(kk == 0), stop=(kk == 1))
                r = work_p.tile([P, D], f32, tag="r")
                if use_biases:
                    nc.vector.tensor_add(out=r[:], in0=v_ps[:], in1=bias_sb[bb_row])
                    if layer == 1:
                        nc.vector.tensor_scalar_max(out=r[:], in0=r[:], scalar1=0.0)
                else:
                    if layer == 1:
                        nc.vector.tensor_scalar_max(out=r[:], in0=v_ps[:], scalar1=0.0)
                    else:
                        nc.vector.tensor_copy(out=r[:], in_=v_ps[:])
                # --- LayerNorm ---
                stats = stat_p.tile([P, 6], f32, tag="stats")
                nc.vector.bn_stats(out=stats[:], in_=r[:])
                mv = stat_p.tile([P, 2], f32, tag="mv")
                nc.vector.bn_aggr(out=mv[:], in_=stats[:])
                nc.scalar.activation(out=mv[:, 1:2], in_=mv[:, 1:2],
                                     func=mybir.ActivationFunctionType.Sqrt,
                                     bias=eps_sb[:], scale=1.0)
                nc.vector.reciprocal(out=mv[:, 1:2], in_=mv[:, 1:2])
                h = work_p.tile([P, D], f32, tag="h")
                nc.vector.tensor_scalar(out=h[:], in0=r[:],
                                        scalar1=mv[:, 0:1], scalar2=mv[:, 1:2],
                                        op0=mybir.AluOpType.subtract,
                                        op1=mybir.AluOpType.mult)
                if use_ln_gb:
                    nc.vector.tensor_mul(out=h[:], in0=h[:], in1=bias_sb[g_row])
                    nc.vector.tensor_add(out=h[:], in0=h[:], in1=bias_sb[b_row])

                if layer == 1:
                    # --- z2 = h @ w2a -> z2_shard ---
                    hT_ps = ps_tr.tile([P, 2, P], f32, tag="uT_ps")
                    for kk in range(2):
                        nc.tensor.transpose(out=hT_ps[:, kk, :],
                                            in_=h[:, kk * P:(kk + 1) * P],
                                            identity=ident_sb[:])
                    hT = work_p.tile([P, 2, P], mm_dt, tag="uT")
                    nc.vector.tensor_copy(out=hT[:], in_=hT_ps[:])
                    z2_ps = ps_mm.tile([P, D], f32, tag="v_ps")
                    for kk in range(2):
                        nc.tensor.matmul(out=z2_ps[:],
                                         lhsT=hT[:, kk, :], rhs=w2a_sb[kk][:],
                                         start=(kk == 0), stop=(kk == 1))
                    z2_sb = work_p.tile([P, D], mm_dt, tag="z2_sb")
                    nc.vector.tensor_copy(out=z2_sb[:], in_=z2_ps[:])
                    nc.sync.dma_start(out=z2_shard[t * P:t * P + valid, :],
                                      in_=z2_sb[:valid, :])
                else:
                    nc.sync.dma_start(out=out_d[t * P:t * P + valid, :],
                                      in_=h[:valid, :])

                if layer == 1 and t in last_tile_of_slice:
                    s = last_tile_of_slice[t]
                    nc.gpsimd.collective_compute(
                        "AllGather", mybir.AluOpType.bypass,
                        replica_groups=groups,
                        ins=[z2_shard[sl_rows[s]:sl_rows[s + 1], :]],
                        outs=[z2_full[glob_off[s]:glob_off[s + 1], :]])

        mp_layer(1)
        mp_layer(2)

    nc.compile()
    return nc


def get_program(Ct, slice_lens, **kw):
    key = (tuple(Ct), tuple(slice_lens), tuple(sorted(kw.items())))
    if key not in _PROGRAM_CACHE:
        _PROGRAM_CACHE[key] = _build_program(Ct, slice_lens, **kw)
    return _PROGRAM_CACHE[key]


# ---------------------------------------------------------------------------
# Entry point
# ---------------------------------------------------------------------------
def kernel_with_results(x_cat, edge_index, emb0, emb1, emb2, emb3,
                        w1a, b1a, w1b, b1b, w2a, b2a, w2b, b2b,
                        ln1_g, ln1_b, ln2_g, ln2_b, trace=False):
    from concourse import bass_utils

    Ct, slice_lens, cmeta, src_meta, dstrel_meta = _prep_meta(
        x_cat, edge_index)

    f32 = np.float32
    embT = np.stack([np.ascontiguousarray(np.asarray(e, f32).T)
                     for e in (emb0, emb1, emb2, emb3)])
    w1a = np.ascontiguousarray(np.asarray(w1a, f32))
    w1b = np.ascontiguousarray(np.asarray(w1b, f32))
    w2a = np.ascontiguousarray(np.asarray(w2a, f32))
    w2b = np.ascontiguousarray(np.asarray(w2b, f32))

    biases = [np.asarray(b, f32) for b in (b1a, b1b, b2a, b2b)]
    lngb = [np.asarray(b, f32) for b in (ln1_g, ln1_b, ln2_g, ln2_b)]
    use_biases = any(np.any(b != 0.0) for b in biases)
    use_ln_gb = (np.any(lngb[0] != 1.0) or np.any(lngb[1] != 0.0)
                 or np.any(lngb[2] != 1.0) or np.any(lngb[3] != 0.0))

    iota_row = np.broadcast_to(np.arange(P, dtype=f32), (P, P)).copy()
    identity = np.eye(P, dtype=f32)

    nc = get_program(Ct, slice_lens, use_biases=use_biases,
                     use_ln_gb=use_ln_gb)

    in_maps = []
    for k in range(NC):
        m = {
            "embT": embT,
            "w1a": w1a, "w1b": w1b, "w2a": w2a, "w2b": w2b,
            "cmeta": cmeta[k],
            "src_meta": src_meta[k],
            "dstrel_meta": dstrel_meta[k],
            "iota_row": iota_row,
            "identity": identity,
        }
        if use_biases or use_ln_gb:
            m["biasrows"] = np.stack(biases + lngb)
        in_maps.append(m)

    res = bass_utils.run_bass_kernel_spmd(nc, in_maps, core_ids=list(range(NC)),
                                          trace=trace)
    out = np.concatenate([r["out"] for r in res.results], axis=0)
    return out.astype(np.float32), res


def kernel(**inputs):
    out, _ = kernel_with_results(**inputs)
    return out



# revision 7
# speedup vs baseline: 1.8059x; 1.8059x over previous
"""Trainium2 Bass kernel for a 2-layer categorical GIN encoder.

Graph: N=100000 nodes, E=1600000 edges, 256-dim features.

    x   = concat_i emb_i[x_cat[:, i]]                  # [N, 256]
    h1  = LN1(relu(relu((x + A x) @ w1a + b1a) @ w1b + b1b))
    out = LN2(relu((h1 + A h1) @ w2a + b2a) @ w2b + bb2b)

where (A x)[d] = sum over edges (s -> d) of x[s].

Strategy (8 NeuronCores, SPMD):
  * Linearity trick: (x + Ax) @ w1a == z + A z with z = x @ w1a, and
    z[n] = sum_i t_i[x_cat[n, i]] where t_i = emb_i @ w1a[64i:64i+64, :].
    The t tables are built on device (bf16), each core builds its shard
    of the bf16 z table by dma_gather of t rows, then shards are
    AllGathered so every core holds the full z table (row = node id).
  * Edges are sorted by destination on the host and sharded by
    destination-node range (12500 nodes per core) -> no collective for
    the aggregation itself.
  * Per-edge gathers of z[src] use the InstDMAGatherAnt ucode (SWDGE
    fixed overhead ~1us is paid per *call*, not per row, so calls are
    batched): edges of a GROUP of 4 destination tiles are gathered with
    4 calls (one per 25000-row source bucket; the int16 index limit
    forces the bucket split).  Slot (p, chunk) holds edge j=chunk*128+p;
    within a (tile, bucket) segment slots are chunk-aligned, padding
    slots gather row 0 of the bucket (idx 0) and carry dst_rel = -1.
  * Aggregation per tile: one-hot S[e, d] = (dst_rel[e] == d) built on
    DVE (bf16 in/out), agg += S.T @ G accumulated in PSUM on the tensor
    engine (bf16 in, fp32 accumulate).  Padding edges select zero.
  * Per-tile MLP on-chip: PE transposes put activations feature-major
    for the bf16 weight matmuls; LayerNorm uses bn_stats/bn_aggr.
    Layer 1 also applies w2a immediately so the second AllGather ships
    z2 = h1 @ w2a; layer 2 repeats the pipeline and writes fp32 output.

Host-side work is limited to index manipulation (edge sort / slotting,
int16 bucket indices) and small layout transforms; all floating-point
math runs on the NeuronCores.
"""

import numpy as np

# ---------------------------------------------------------------------------
# Problem constants (hardcoded per contest contract)
# ---------------------------------------------------------------------------
N = 100000        # nodes
E = 1600000       # edges
D = 256           # feature dim (in = hidden = out)
EMB = 64          # per-field embedding dim
V = 1000          # categories per field
NF = 4            # categorical fields
NC = 8            # NeuronCores
P = 128           # partitions
LN_EPS = 1e-5

NSH = N // NC             # nodes per core (12500)
NT = (NSH + P - 1) // P   # node tiles per core (98)
LAST_VALID = NSH - (NT - 1) * P  # valid rows in last tile (84)

NBUK = 4                  # source buckets (int16 index limit)
BUK = N // NBUK           # 25000 rows per bucket
GSZ = 4                   # dst tiles per gather group
CGT = 4                   # tiles per phase-C gather call


def _wrap_idx(flat):
    """[L] int16 (L % 16 == 0) -> [128, L//16] wrapped + replicated."""
    w = flat.reshape(-1, 16).T.copy()           # [16, L//16]
    return np.tile(w, (8, 1))                   # [128, L//16]


# ---------------------------------------------------------------------------
# Host-side preprocessing: shard + sort edges, build per-core metadata
# ---------------------------------------------------------------------------
def _prep_meta(x_cat, edge_index):
    """Returns (Cb, cmeta16, idx_meta[NC], drel_meta[NC]).

    Cb[t][b] = chunks for tile t bucket b (max over cores; the single
    SPMD program is shape-uniform).  idx_meta is the wrapped int16 index
    stream for the per-(group,bucket) dma_gather calls; drel_meta the
    bf16 dst_rel value per (partition, global chunk column).
    """
    import ml_dtypes

    src = np.asarray(edge_index[0], dtype=np.int64)
    dst = np.asarray(edge_index[1], dtype=np.int64)

    order = np.argsort(dst, kind="stable")
    dst_s = dst[order]
    src_s = src[order]
    bounds = np.searchsorted(dst_s, np.arange(NC + 1) * NSH)

    per_core = []
    counts_tb = np.zeros((NC, NT * NBUK), dtype=np.int64)
    for k in range(NC):
        lo, hi = bounds[k], bounds[k + 1]
        d_k = dst_s[lo:hi] - k * NSH
        s_k = src_s[lo:hi]
        t_k = d_k // P
        rel = (d_k - t_k * P).astype(np.int64)
        b_k = s_k // BUK
        key = t_k * NBUK + b_k
        o2 = np.argsort(key, kind="stable")
        key = key[o2]
        counts = np.bincount(key, minlength=NT * NBUK)
        counts_tb[k] = counts
        per_core.append((s_k[o2], rel[o2], key, counts))

    Cb = -(-counts_tb.max(axis=0) // P).reshape(NT, NBUK)  # [NT, NBUK] chunks
    Cb = np.maximum(Cb, (counts_tb.max(axis=0).reshape(NT, NBUK) > 0))

    # global chunk column layout: group-major, bucket-major inside group
    n_groups = -(-NT // GSZ)
    col_start = np.zeros((NT, NBUK), dtype=np.int64)
    call_cols = []           # per group: [(b, col_lo, col_hi)]
    cc = 0
    for g in range(n_groups):
        tiles = range(g * GSZ, min((g + 1) * GSZ, NT))
        calls = []
        for b in range(NBUK):
            lo = cc
            for t in tiles:
                col_start[t, b] = cc
                cc += Cb[t, b]
            # SWDGE descriptor ring caps one call at 1024 indices (8 chunks)
            for s in range(lo, cc, 8):
                calls.append((b, s, min(s + 8, cc)))
        call_cols.append(calls)
    CT = int(cc)

    # tile -> list of (col_lo, col_hi) chunk ranges (per bucket)
    tile_ranges = [[(int(col_start[t, b]), int(col_start[t, b] + Cb[t, b]))
                    for b in range(NBUK) if Cb[t, b] > 0] for t in range(NT)]

    idx_meta, drel_meta = [], []
    for k in range(NC):
        s_k, rel_k, key_k, counts = per_core[k]
        starts = np.zeros(NT * NBUK, dtype=np.int64)
        np.cumsum(counts[:-1], out=starts[1:])
        j_seg = np.arange(len(s_k)) - starts[key_k]     # rank within (t,b)
        col = col_start.reshape(-1)[key_k] + j_seg // P
        slot = col * P + (j_seg % P)                    # global flat slot

        idxflat = np.zeros(CT * P, dtype=np.int16)
        drelflat = np.full(CT * P, -1.0, dtype=np.float32)
        idxflat[slot] = (s_k - (key_k % NBUK) * BUK).astype(np.int16)
        drelflat[slot] = rel_k

        # wrapped per-call idx stream (calls are column ranges -> flat
        # slot ranges, already contiguous in group-major layout)
        idx_meta.append(_wrap_idx(idxflat))
        drel_meta.append(
            drelflat.reshape(CT, P).T.astype(ml_dtypes.bfloat16).copy())

    # phase-C index stream: slot (tile_in_call*NF + f)*128 + p
    x_cat = np.asarray(x_cat, dtype=np.int64)
    NTC = -(-NT // CGT)
    cmflat = np.zeros(NT * NF * P, dtype=np.int16)
    cmeta16 = []
    for k in range(NC):
        xc = x_cat[k * NSH:(k + 1) * NSH]               # [NSH, NF]
        rows = (xc + np.arange(NF)[None, :] * V).astype(np.int16)
        cm = cmflat.copy().reshape(NT, NF, P)
        for t in range(NT):
            v = min(P, NSH - t * P)
            cm[t, :, :v] = rows[t * P:t * P + v].T
        cmeta16.append(_wrap_idx(cm.reshape(-1)))

    Cb_list = [[int(Cb[t, b]) for b in range(NBUK)] for t in range(NT)]
    return Cb_list, call_cols, tile_ranges, cmeta16, idx_meta, drel_meta


# ---------------------------------------------------------------------------
# Device program
# ---------------------------------------------------------------------------
_PROGRAM_CACHE = {}


def _build_program(Cb, call_cols, tile_ranges,
                   use_biases=False, use_ln_gb=False, num_cores=NC):
    """Build + compile the SPMD Bass program."""
    import concourse.bacc as bacc
    import concourse.bass as bass
    import concourse.tile as tile
    from concourse import mybir

    f32 = mybir.dt.float32
    i16 = mybir.dt.int16
    bf16 = mybir.dt.bfloat16

    nc = bacc.Bacc("TRN2", target_bir_lowering=False, debug=False,
                   num_devices=num_cores, num_swdge_queues=4)

    CT = call_cols[-1][-1][2]
    n_groups = len(call_cols)
    NTC = -(-NT // CGT)
    Cg_max = max(calls[-1][2] - calls[0][1] for calls in call_cols)

    # ---- external I/O ----
    embT_d = nc.dram_tensor("embT", [NF, EMB, V], f32, kind="ExternalInput")
    w1a_d = nc.dram_tensor("w1a", [D, D], f32, kind="ExternalInput")
    w1b_d = nc.dram_tensor("w1b", [D, D], f32, kind="ExternalInput")
    w2a_d = nc.dram_tensor("w2a", [D, D], f32, kind="ExternalInput")
    w2b_d = nc.dram_tensor("w2b", [D, D], f32, kind="ExternalInput")
    cmeta_d = nc.dram_tensor("cmeta16", [128, NT * NF * P // 16], i16,
                             kind="ExternalInput")
    idx_d = nc.dram_tensor("idx_meta", [128, CT * P // 16], i16,
                           kind="ExternalInput")
    drel_d = nc.dram_tensor("drel_meta", [P, CT], bf16, kind="ExternalInput")
    iota_d = nc.dram_tensor("iota_row", [P, P], bf16, kind="ExternalInput")
    ident_d = nc.dram_tensor("identity", [P, P], f32, kind="ExternalInput")
    bias_d = None
    if use_biases or use_ln_gb:
        # rows: b1a, b1b, b2a, b2b, ln1_g, ln1_b, ln2_g, ln2_b
        bias_d = nc.dram_tensor("biasrows", [8, D], f32, kind="ExternalInput")
    out_d = nc.dram_tensor("out", [NSH, D], f32, kind="ExternalOutput")

    groups = [list(range(num_cores))]

    from contextlib import ExitStack

    qctr = [0]

    def next_q():
        q = qctr[0] % 4
        qctr[0] += 1
        return q

    with tile.TileContext(nc) as tc, ExitStack() as ctx:
        singles = ctx.enter_context(tc.tile_pool(name="singles", bufs=1))
        dram = ctx.enter_context(tc.tile_pool(name="dram", bufs=1, space="DRAM"))
        meta_p = ctx.enter_context(tc.tile_pool(name="meta", bufs=4))
        gath_p = ctx.enter_context(tc.tile_pool(name="gath", bufs=2))
        g4_p = ctx.enter_context(tc.tile_pool(name="g4", bufs=2))
        sel_p = ctx.enter_context(tc.tile_pool(name="sel", bufs=2))
        work_p = ctx.enter_context(tc.tile_pool(name="work", bufs=4))
        stat_p = ctx.enter_context(tc.tile_pool(name="stat", bufs=4))
        ps_agg = ctx.enter_context(tc.tile_pool(name="ps_agg", bufs=3, space="PSUM"))
        ps_tr = ctx.enter_context(tc.tile_pool(name="ps_tr", bufs=2, space="PSUM"))
        ps_mm = ctx.enter_context(tc.tile_pool(name="ps_mm", bufs=2, space="PSUM"))

        # ---- internal DRAM tables ----
        t_dram = dram.tile([NF * V, D], bf16)
        z_shard = dram.tile([NSH, D], bf16)
        z_full = dram.tile([N, D], bf16, addr_space="Shared")
        z2_shard = dram.tile([NSH, D], bf16)
        z2_full = dram.tile([N, D], bf16, addr_space="Shared")

        # ---- persistent SBUF constants ----
        iota_sb = singles.tile([P, P], bf16)
        nc.sync.dma_start(out=iota_sb[:], in_=iota_d[:])
        ident_sb = singles.tile([P, P], f32)
        nc.sync.dma_start(out=ident_sb[:], in_=ident_d[:])
        eps_sb = singles.tile([P, 1], f32)
        nc.vector.memset(eps_sb[:], LN_EPS)

        def load_w(dram_t, name):
            tiles = []
            for kk in range(2):
                w_sb = singles.tile([P, D], f32, name=f"{name}_{kk}")
                nc.sync.dma_start(out=w_sb[:], in_=dram_t[kk * P:(kk + 1) * P, :])
                tiles.append(w_sb)
            return tiles

        w1a_rows = []
        for f in range(NF):
            w1a_r = singles.tile([EMB, D], f32, name=f"w1a_r{f}")
            nc.sync.dma_start(out=w1a_r[:], in_=w1a_d[f * EMB:(f + 1) * EMB, :])
            w1a_rows.append(w1a_r)

        def round_w(tiles, name):
            out = []
            for kk, w_sb in enumerate(tiles):
                w_r = singles.tile([P, D], bf16, name=f"{name}r_{kk}")
                nc.vector.tensor_copy(out=w_r[:], in_=w_sb[:])
                out.append(w_r)
            return out

        w1b_sb = round_w(load_w(w1b_d, "w1b"), "w1b")
        w2a_sb = round_w(load_w(w2a_d, "w2a"), "w2a")
        w2b_sb = round_w(load_w(w2b_d, "w2b"), "w2b")

        bias_sb = None
        if bias_d is not None:
            bias_tile = singles.tile([P, 8, D], f32)
            for r in range(8):
                nc.sync.dma_start(
                    out=bias_tile[:, r, :],
                    in_=bias_d[r].unsqueeze(0).to_broadcast([P, D]))
            bias_sb = [bias_tile[:, r, :] for r in range(8)]

        # =================================================================
        # Phase B: t tables  t[f] = emb_f @ w1a[64f:64f+64, :]   -> t_dram
        # =================================================================
        MT = 125  # 1000 = 8 * 125
        embT_p = ctx.enter_context(tc.tile_pool(name="embT_p", bufs=1))
        for f in range(NF):
            embT_sb = embT_p.tile([EMB, V], f32, tag="embT")
            nc.sync.dma_start(out=embT_sb[:], in_=embT_d[f])
            w_rows = w1a_rows[f][:]
            for j in range(V // MT):
                t_ps = ps_mm.tile([MT, D], f32, tag="tps", bufs=1)
                nc.tensor.matmul(out=t_ps[:],
                                 lhsT=embT_sb[:, j * MT:(j + 1) * MT],
                                 rhs=w_rows, start=True, stop=True)
                t_sb = work_p.tile([MT, D], bf16, tag="tsb")
                nc.vector.tensor_copy(out=t_sb[:], in_=t_ps[:])
                nc.sync.dma_start(
                    out=t_dram[f * V + j * MT:f * V + (j + 1) * MT, :],
                    in_=t_sb[:])

        # =================================================================
        # Phase C: z shard  z[n] = sum_f t[cmeta[n, f]]          -> z_shard
        # =================================================================
        for gg in range(NTC):
            t0 = gg * CGT
            ntl = min(CGT, NT - t0)
            L = ntl * NF * P
            cm = meta_p.tile([128, CGT * NF * P // 16], i16, tag="cm")
            o16 = t0 * NF * P // 16
            nc.sync.dma_start(out=cm[:, :L // 16],
                              in_=cmeta_d[:, o16:o16 + L // 16])
            g4 = g4_p.tile([P, CGT * NF, D], bf16, tag="g4")
            for s in range(0, ntl * NF, 8):
                e = min(s + 8, ntl * NF)
                nc.gpsimd.dma_gather(
                    out_ap=g4[:, s:e, :], in_ap=t_dram[:, :],
                    idxs_ap=cm[:, s * 8:e * 8],
                    num_idxs=(e - s) * P, num_idxs_reg=(e - s) * P,
                    elem_size=D, queue_num=next_q())
            for tt in range(ntl):
                t = t0 + tt
                valid = LAST_VALID if t == NT - 1 else P
                t01 = work_p.tile([P, 2, D], f32, tag="t01")
                nc.vector.tensor_add(out=t01[:],
                                     in0=g4[:, tt * NF:tt * NF + 2, :],
                                     in1=g4[:, tt * NF + 2:tt * NF + 4, :])
                z_t = work_p.tile([P, D], bf16, tag="z_t")
                nc.vector.tensor_add(out=z_t[:], in0=t01[:, 0, :],
                                     in1=t01[:, 1, :])
                nc.sync.dma_start(out=z_shard[t * P:t * P + valid, :],
                                  in_=z_t[:valid, :])

        nc.gpsimd.collective_compute(
            "AllGather", mybir.AluOpType.bypass, replica_groups=groups,
            ins=[z_shard[:]], outs=[z_full[:]])

        # =================================================================
        # Phases D/E: message passing + MLP layers
        # =================================================================
        def mp_layer(layer):
            tab_full = z_full if layer == 1 else z2_full
            tab_own = z_shard if layer == 1 else z2_shard
            wb_sb = w1b_sb if layer == 1 else w2b_sb
            ba_row, bb_row = (0, 1) if layer == 1 else (2, 3)
            g_row, b_row = (4, 5) if layer == 1 else (6, 7)

            for g in range(n_groups):
                calls = call_cols[g]
                g_lo = calls[0][1]
                g_hi = calls[-1][2]
                Cg = g_hi - g_lo
                if Cg == 0:
                    continue
                # --- metadata ---
                idx_sb = meta_p.tile([128, Cg_max * 8], i16, tag="idx")
                nc.sync.dma_start(
                    out=idx_sb[:, :Cg * 8],
                    in_=idx_d[:, g_lo * 8:g_hi * 8])
                drel = meta_p.tile([P, Cg_max], bf16, tag="drel")
                nc.sync.dma_start(out=drel[:, :Cg],
                                  in_=drel_d[:, g_lo:g_hi])
                # --- gather: one call per bucket ---
                G = gath_p.tile([P, Cg_max, D], bf16, tag="G")
                for b, c_lo, c_hi in calls:
                    nb = c_hi - c_lo
                    if nb == 0:
                        continue
                    nc.gpsimd.dma_gather(
                        out_ap=G[:, c_lo - g_lo:c_hi - g_lo, :],
                        in_ap=tab_full[b * BUK:(b + 1) * BUK, :],
                        idxs_ap=idx_sb[:, (c_lo - g_lo) * 8:(c_hi - g_lo) * 8],
                        num_idxs=nb * P, num_idxs_reg=nb * P, elem_size=D,
                        queue_num=next_q())
                # --- selection matrix for the whole group ---
                S = sel_p.tile([P, Cg_max, P], bf16, tag="S")
                nc.vector.tensor_tensor(
                    out=S[:, :Cg, :],
                    in0=drel[:, :Cg].unsqueeze(2).to_broadcast([P, Cg, P]),
                    in1=iota_sb[:].unsqueeze(1).to_broadcast([P, Cg, P]),
                    op=mybir.AluOpType.is_equal)

                for t in range(g * GSZ, min((g + 1) * GSZ, NT)):
                    valid = LAST_VALID if t == NT - 1 else P
                    ranges = tile_ranges[t]
                    ncols = sum(hi - lo for lo, hi in ranges)
                    # --- aggregate: agg[d, :] += S[:, c, d].T @ G[:, c, :] ---
                    agg_ps = ps_agg.tile([P, D], f32, tag="agg")
                    ci = 0
                    for lo, hi in ranges:
                        for c in range(lo - g_lo, hi - g_lo):
                            nc.tensor.matmul(out=agg_ps[:],
                                             lhsT=S[:, c, :], rhs=G[:, c, :],
                                             start=(ci == 0),
                                             stop=(ci == ncols - 1))
                            ci += 1
                    # --- u = relu(z_own + agg (+ba)) ---
                    zown = work_p.tile([P, D], bf16, tag="zown")
                    if valid < P:
                        nc.vector.memset(zown[:], 0.0)
                    nc.sync.dma_start(out=zown[:valid, :],
                                      in_=tab_own[t * P:t * P + valid, :])
                    u = work_p.tile([P, D], f32, tag="u")
                    nc.vector.tensor_add(out=u[:], in0=agg_ps[:], in1=zown[:])
                    if use_biases:
                        nc.vector.tensor_add(out=u[:], in0=u[:],
                                             in1=bias_sb[ba_row])
                    nc.vector.tensor_scalar_max(out=u[:], in0=u[:], scalar1=0.0)
                    # --- v = u @ wb (+bb) ---
                    uT_ps = ps_tr.tile([P, 2, P], f32, tag="uT_ps")
                    for kk in range(2):
                        nc.tensor.transpose(out=uT_ps[:, kk, :],
                                            in_=u[:, kk * P:(kk + 1) * P],
                                            identity=ident_sb[:])
                    uT = work_p.tile([P, 2, P], bf16, tag="uT")
                    nc.vector.tensor_copy(out=uT[:], in_=uT_ps[:])
                    v_ps = ps_mm.tile([P, D], f32, tag="v_ps")
                    for kk in range(2):
                        nc.tensor.matmul(out=v_ps[:],
                                         lhsT=uT[:, kk, :], rhs=wb_sb[kk][:],
                                         start=(kk == 0), stop=(kk == 1))
                    r = work_p.tile([P, D], f32, tag="r")
                    if use_biases:
                        nc.vector.tensor_add(out=r[:], in0=v_ps[:],
                                             in1=bias_sb[bb_row])
                        if layer == 1:
                            nc.vector.tensor_scalar_max(out=r[:], in0=r[:],
                                                        scalar1=0.0)
                    else:
                        if layer == 1:
                            nc.vector.tensor_scalar_max(out=r[:], in0=v_ps[:],
                                                        scalar1=0.0)
                        else:
                            nc.vector.tensor_copy(out=r[:], in_=v_ps[:])
                    # --- LayerNorm ---
                    stats = stat_p.tile([P, 6], f32, tag="stats")
                    nc.vector.bn_stats(out=stats[:], in_=r[:])
                    mv = stat_p.tile([P, 2], f32, tag="mv")
                    nc.vector.bn_aggr(out=mv[:], in_=stats[:])
                    nc.scalar.activation(out=mv[:, 1:2], in_=mv[:, 1:2],
                                         func=mybir.ActivationFunctionType.Sqrt,
                                         bias=eps_sb[:], scale=1.0)
                    nc.vector.reciprocal(out=mv[:, 1:2], in_=mv[:, 1:2])
                    h = work_p.tile([P, D], f32, tag="h")
                    nc.vector.tensor_scalar(out=h[:], in0=r[:],
                                            scalar1=mv[:, 0:1],
                                            scalar2=mv[:, 1:2],
                                            op0=mybir.AluOpType.subtract,
                                            op1=mybir.AluOpType.mult)
                    if use_ln_gb:
                        nc.vector.tensor_mul(out=h[:], in0=h[:],
                                             in1=bias_sb[g_row])
                        nc.vector.tensor_add(out=h[:], in0=h[:],
                                             in1=bias_sb[b_row])

                    if layer == 1:
                        # --- z2 = h @ w2a -> z2_shard ---
                        hT_ps = ps_tr.tile([P, 2, P], f32, tag="uT_ps")
                        for kk in range(2):
                            nc.tensor.transpose(out=hT_ps[:, kk, :],
                                                in_=h[:, kk * P:(kk + 1) * P],
                                                identity=ident_sb[:])
                        hT = work_p.tile([P, 2, P], bf16, tag="uT")
                        nc.vector.tensor_copy(out=hT[:], in_=hT_ps[:])
                        z2_ps = ps_mm.tile([P, D], f32, tag="v_ps")
                        for kk in range(2):
                            nc.tensor.matmul(out=z2_ps[:],
                                             lhsT=hT[:, kk, :],
                                             rhs=w2a_sb[kk][:],
                                             start=(kk == 0), stop=(kk == 1))
                        z2_sb = work_p.tile([P, D], bf16, tag="z2_sb")
                        nc.vector.tensor_copy(out=z2_sb[:], in_=z2_ps[:])
                        nc.sync.dma_start(
                            out=z2_shard[t * P:t * P + valid, :],
                            in_=z2_sb[:valid, :])
                    else:
                        nc.sync.dma_start(out=out_d[t * P:t * P + valid, :],
                                          in_=h[:valid, :])

            if layer == 1:
                nc.gpsimd.collective_compute(
                    "AllGather", mybir.AluOpType.bypass,
                    replica_groups=groups,
                    ins=[z2_shard[:]], outs=[z2_full[:]])

        mp_layer(1)
        mp_layer(2)

    nc.compile()
    return nc


def get_program(Cb, call_cols, tile_ranges, **kw):
    key = (tuple(tuple(c) for c in Cb), tuple(sorted(kw.items())))
    if key not in _PROGRAM_CACHE:
        _PROGRAM_CACHE[key] = _build_program(Cb, call_cols, tile_ranges, **kw)
    return _PROGRAM_CACHE[key]


# ---------------------------------------------------------------------------
# Entry point
# ---------------------------------------------------------------------------
def kernel_with_results(x_cat, edge_index, emb0, emb1, emb2, emb3,
                        w1a, b1a, w1b, b1b, w2a, b2a, w2b, b2b,
                        ln1_g, ln1_b, ln2_g, ln2_b, trace=False):
    import ml_dtypes
    from concourse import bass_utils

    Cb, call_cols, tile_ranges, cmeta16, idx_meta, drel_meta = _prep_meta(
        x_cat, edge_index)

    f32 = np.float32
    embT = np.stack([np.ascontiguousarray(np.asarray(e, f32).T)
                     for e in (emb0, emb1, emb2, emb3)])
    w1a = np.ascontiguousarray(np.asarray(w1a, f32))
    w1b = np.ascontiguousarray(np.asarray(w1b, f32))
    w2a = np.ascontiguousarray(np.asarray(w2a, f32))
    w2b = np.ascontiguousarray(np.asarray(w2b, f32))

    biases = [np.asarray(b, f32) for b in (b1a, b1b, b2a, b2b)]
    lngb = [np.asarray(b, f32) for b in (ln1_g, ln1_b, ln2_g, ln2_b)]
    use_biases = any(np.any(b != 0.0) for b in biases)
    use_ln_gb = (np.any(lngb[0] != 1.0) or np.any(lngb[1] != 0.0)
                 or np.any(lngb[2] != 1.0) or np.any(lngb[3] != 0.0))

    iota_row = np.broadcast_to(
        np.arange(P).astype(ml_dtypes.bfloat16), (P, P)).copy()
    identity = np.eye(P, dtype=f32)

    nc = get_program(Cb, call_cols, tile_ranges, use_biases=use_biases,
                     use_ln_gb=use_ln_gb)

    in_maps = []
    for k in range(NC):
        m = {
            "embT": embT,
            "w1a": w1a, "w1b": w1b, "w2a": w2a, "w2b": w2b,
            "cmeta16": cmeta16[k],
            "idx_meta": idx_meta[k],
            "drel_meta": drel_meta[k],
            "iota_row": iota_row,
            "identity": identity,
        }
        if use_biases or use_ln_gb:
            m["biasrows"] = np.stack(biases + lngb)
        in_maps.append(m)

    res = bass_utils.run_bass_kernel_spmd(nc, in_maps, core_ids=list(range(NC)),
                                          trace=trace)
    out = np.concatenate([r["out"] for r in res.results], axis=0)
    return out.astype(np.float32), res


def kernel(**inputs):
    out, _ = kernel_with_results(**inputs)
    return out
